# revision 1
# baseline (speedup 1.0000x reference)
"""Trainium2 Bass kernel for nn_LlamaEmbeddingClassifier.

Model: 2-layer Llama (D=512, 8 heads x 64, HID=1408, RoPE, RMSNorm) scoring
B=4 prompts against NLAB=5 label continuations (LBL=4 tokens) with an
lm_head over V=128000.

Strategy (8 NeuronCores, single SPMD launch):
  - The 5 label variants of each batch row share a 508-token prefix; causal
    attention means prefix activations are label-independent. Each core runs
    the transformer over a packed 528-token sequence
    [508 prefix | 5 x 4-token suffixes] with a custom attention mask.
    Core c handles batch row (c % 4); cores 4-7 duplicate 0-3 (the SPMD
    program is uniform; their phase-1 output is ignored).
  - Only 16 positions feed the classifier (pos 507 + first 3 tokens of each
    suffix), so layer 2 runs full K/V projections but a THIN query path
    (16 queries) for attention/MLP — layer-2 cost collapses to ~K/V cost.
  - Activations live TRANSPOSED in SBUF as [D(partitions), tokens] so every
    projection uses the weight matrices as-given for lhsT and never needs an
    on-device transpose. Scores are computed transposed [k, q]; softmax is
    un-normalized exp (scores are bounded ~|1.5|, so no max-subtraction),
    with the row-sum obtained free via a ones-column appended to V.
    Fully-masked causal blocks are skipped.
  - The 16 final hidden rows per batch are AllGathered (tiny) so every core
    holds all 80 scoring rows.
  - lm_head [512,128000] is the memory bottleneck: vocab-sharded 8 ways,
    streamed as scaled fp8-e4m3 in DoubleRow perf mode (halves both HBM
    traffic and PE cycles; the fp8 noise averages out in the logsumexp);
    each core computes partial sumexp over its 16000 columns.  Label-token
    logits come from a tiny bf16 side matmul against the 20 selected
    lm_head columns, so the accuracy-critical path avoids fp8.  Host
    combines: logsumexp across shards, lp = sel_logit - lse, summed per
    (batch,label).
  - Other matmuls bf16 with fp32 PSUM accumulation; norms/softmax sums fp32.
"""

import math
import os
import sys
from contextlib import ExitStack

for _p in ("/opt/trn_rl_repo", "/root/.axon_site/_ro/trn_rl_repo"):
    if os.path.isdir(_p) and _p not in sys.path:
        sys.path.insert(0, _p)

import ml_dtypes
import numpy as np

import concourse.bass as bass
import concourse.tile as tile
from concourse import bacc, mybir
from concourse.bass_utils import run_bass_kernel_spmd

BF16 = ml_dtypes.bfloat16

# Problem dims (hardcoded per contract)
V, D, NH, NL, HID = 128000, 512, 8, 2, 1408
HD, HALF = 64, 32
B, T, NLAB, LBL = 4, 508, 5, 4
EPS = 1e-5
NCORES = 8
SUF = NLAB * LBL            # 20 suffix tokens
NTOK = T + SUF              # 528 packed tokens
KT = D // 128               # 4 K-tiles over D
TT = (NTOK + 127) // 128    # 5 token tiles (last has 16 rows)
HT = HID // 128             # 11 tiles over HID
NROW = B * NLAB * LBL       # 80 scoring rows
NSEL = NLAB * LBL           # 20 selected lm_head columns
NQ = 1 + NLAB * (LBL - 1)   # 16 thin-path query positions
VSH = V // NCORES           # 16000 vocab shard per core
VCH = 500                   # vocab chunk per matmul sweep
NVCH = VSH // VCH           # 32 chunks
QC = 264                    # q chunk (2 chunks of 264 = 528)
LMH_SCALE = 32.0            # host premultiply of fp8 lm_head (fp8 dynamic range)
HS_SCALE = 4.0              # device premultiply of fp8 hs copies

# causal block structure for the full layer: q-chunk 0 (q<264) only sees
# k-tiles 0,1; q-chunk 1 sees all.  Mask-multiply only where partial.
CH_MTS = {0: (0, 1), 1: (0, 1, 2, 3, 4)}
# mask only the partial blocks: ch0 triangles for mt 0,1; ch1 edges for mt>=2
MASK_SLICES = {0: (0, QC), 1: (0, QC),
               2: (QC, NTOK), 3: (QC, NTOK), 4: (QC, NTOK)}

_CACHE = {}
# NB: the "batched" thin path (multiple matmuls writing one PSUM tile at
# different free offsets) crashes real HW (NRT_EXEC_UNIT_UNRECOVERABLE) even
# though CoreSim/walrus accept it -- matmul PSUM outputs must start at the
# tile's offset 0.  Keep unbatched.
BATCH_THIN = os.environ.get("K_BATCH_THIN", "0") == "1"
USE_FP8 = os.environ.get("K_FP8", "1") == "1"


def _tok_rows(tt):
    return min(128, NTOK - tt * 128)


def build_nc(use_collective=True):
    nc = bacc.Bacc("TRN2", num_devices=NCORES)
    f32, bf16 = mybir.dt.float32, mybir.dt.bfloat16

    # ---- I/O ----
    x0T = nc.dram_tensor("x0T", [D, NTOK], f32, kind="ExternalInput")
    wq = nc.dram_tensor("wq", [NL, D, D], bf16, kind="ExternalInput")
    wk = nc.dram_tensor("wk", [NL, D, D], bf16, kind="ExternalInput")
    wv = nc.dram_tensor("wv", [NL, D, D], bf16, kind="ExternalInput")
    wo = nc.dram_tensor("wo", [NL, D, D], bf16, kind="ExternalInput")
    w1 = nc.dram_tensor("w1", [NL, D, HID], bf16, kind="ExternalInput")
    w3 = nc.dram_tensor("w3", [NL, D, HID], bf16, kind="ExternalInput")
    w2 = nc.dram_tensor("w2", [NL, HID, D], bf16, kind="ExternalInput")
    ctab = nc.dram_tensor("ctab", [128, NTOK], bf16, kind="ExternalInput")
    stab = nc.dram_tensor("stab", [128, NTOK], bf16, kind="ExternalInput")
    cqtab = nc.dram_tensor("cqtab", [128, NQ], bf16, kind="ExternalInput")
    sqtab = nc.dram_tensor("sqtab", [128, NQ], bf16, kind="ExternalInput")
    permd = nc.dram_tensor("permd", [128, 128], bf16, kind="ExternalInput")
    maskd = nc.dram_tensor("maskd", [TT, 128, NTOK], bf16, kind="ExternalInput")
    maskq = nc.dram_tensor("maskq", [TT, 128, NH * NQ], bf16, kind="ExternalInput")
    # lm_head shard in fp8 (DoubleRow layout: [pass, 128, 2, V]), scaled by
    # LMH_SCALE on host; compensated in the exp scale.
    fp8 = mybir.dt.float8e4
    if USE_FP8:
        lmh = nc.dram_tensor("lmh", [2, 128, 2, VSH], fp8, kind="ExternalInput")
    else:
        lmh = nc.dram_tensor("lmh", [D, VSH], bf16, kind="ExternalInput")
    lmsel = nc.dram_tensor("lmsel", [D, NSEL], bf16, kind="ExternalInput")

    se_out = nc.dram_tensor("se_out", [NROW, NVCH], f32, kind="ExternalOutput")
    sel_out = nc.dram_tensor("sel_out", [NROW, NSEL], f32, kind="ExternalOutput")

    with tile.TileContext(nc) as tc, ExitStack() as ctx:
        consts = ctx.enter_context(tc.tile_pool(name="consts", bufs=1))
        wpool = ctx.enter_context(tc.tile_pool(name="weights", bufs=1))
        acts = ctx.enter_context(tc.tile_pool(name="acts", bufs=1))
        scr = ctx.enter_context(tc.tile_pool(name="scratch", bufs=3))
        ppool = ctx.enter_context(tc.tile_pool(name="p", bufs=2))
        psum = ctx.enter_context(tc.tile_pool(name="psum", bufs=6, space="PSUM"))
        psmall = ctx.enter_context(tc.tile_pool(name="psmall", bufs=2, space="PSUM"))
        lpool = ctx.enter_context(tc.tile_pool(name="lmh", bufs=20))
        dram = ctx.enter_context(tc.tile_pool(name="dram", bufs=1, space="DRAM"))
        if True:
            # ---- constants ----
            ones_col = consts.tile([128, 1], bf16)       # rms partition-reduce lhsT
            nc.vector.memset(ones_col, 1.0)
            eps_sb = consts.tile([1, 1], f32)
            nc.vector.memset(eps_sb, float(EPS))
            C128 = consts.tile([128, NTOK], bf16)
            S128 = consts.tile([128, NTOK], bf16)
            nc.sync.dma_start(out=C128[:], in_=ctab[:])
            nc.sync.dma_start(out=S128[:], in_=stab[:])
            Cq = consts.tile([128, NQ], bf16)
            Sq = consts.tile([128, NQ], bf16)
            nc.sync.dma_start(out=Cq[:], in_=cqtab[:])
            nc.sync.dma_start(out=Sq[:], in_=sqtab[:])
            perm = consts.tile([128, 128], bf16)
            nc.sync.dma_start(out=perm[:], in_=permd[:])
            # residual stream hT [D, tok] fp32 -- loaded first (first consumer)
            h = acts.tile([128, KT, NTOK], f32)
            nc.sync.dma_start(out=h[:], in_=x0T.rearrange("(k p) q -> p k q", p=128))
            mask = consts.tile([128, TT, NTOK], bf16)
            nc.sync.dma_start(out=mask[:], in_=maskd.rearrange("t p q -> p t q"))
            mq = consts.tile([128, TT, NH * NQ], bf16)
            nc.sync.dma_start(out=mq[:], in_=maskq.rearrange("t p q -> p t q"))

            # v with interleaved ones column: [tok, 8 heads x (64 v | 1 one)]
            v_aug = acts.tile([128, TT, NH * (HD + 1)], bf16)
            nc.vector.memset(
                v_aug.rearrange("p t (h c) -> p t h c", c=HD + 1)[:, :, :, HD], 1.0
            )

            qT = acts.tile([128, KT, NTOK], bf16, name="qT")
            kTt = acts.tile([128, KT, NTOK], bf16, name="kTt")
            oT = acts.tile([128, KT, NTOK], bf16, name="oT")
            g1 = acts.tile([128, HT, NTOK], bf16, name="g1")
            hq = acts.tile([128, KT, NQ], f32, name="hq")
            qTq = acts.tile([128, KT, NQ], bf16, name="qTq")
            oTq = acts.tile([128, KT, NQ], bf16, name="oTq")
            g1q = acts.tile([128, HT, NQ], bf16, name="g1q")

            def rms(src, dest, n, chunks):
                """dest (bf16) = src * rsqrt(mean_D(src^2)+eps); [128,KT,n]"""
                sq = scr.tile([128, KT, n], bf16, name=f"sq{n}", bufs=1)
                for kt in range(KT):
                    nc.scalar.activation(out=sq[:, kt, :], in_=src[:, kt, :],
                                         func=mybir.ActivationFunctionType.Square)
                rstd = scr.tile([1, n], f32, name=f"rstd{n}")
                for c0, c1 in chunks:
                    ss = psmall.tile([1, c1 - c0], f32, name="ss", tag="small")
                    for kt in range(KT):
                        nc.tensor.matmul(ss[:], ones_col[:], sq[:, kt, c0:c1],
                                         start=(kt == 0), stop=(kt == KT - 1))
                    nc.scalar.activation(out=rstd[:, c0:c1], in_=ss[:],
                                         func=mybir.ActivationFunctionType.Sqrt,
                                         scale=1.0 / D, bias=eps_sb[:])
                nc.vector.reciprocal(out=rstd[:], in_=rstd[:])
                rb = scr.tile([128, n], f32, name=f"rms_rb{n}")
                nc.gpsimd.partition_broadcast(rb[:], rstd[:])
                for kt in range(KT):
                    nc.vector.tensor_mul(out=dest[:, kt, :], in0=src[:, kt, :],
                                         in1=rb[:])

            def proj_T(dest, w_sb, xn, chunks, rope, ctb=None, stb=None):
                """dest[Dout, n] = (xn @ W).T via lhsT=W; optional RoPE."""
                for mt in range(KT):
                    for c0, c1 in chunks:
                        n = c1 - c0
                        ps = psum.tile([128, QC], f32, name="proj_ps", tag="mm")
                        for kt in range(KT):
                            nc.tensor.matmul(
                                ps[:, :n], w_sb[:, kt, mt * 128:(mt + 1) * 128],
                                xn[:, kt, c0:c1],
                                start=(kt == 0), stop=(kt == KT - 1))
                        if not rope:
                            nc.scalar.copy(out=dest[:, mt, c0:c1], in_=ps[:, :n])
                            continue
                        # RoPE: out = raw*C + (perm @ raw)*S
                        raw = scr.tile([128, QC], bf16, name="rope_raw")
                        nc.vector.tensor_copy(out=raw[:, :n], in_=ps[:, :n])
                        sw_ps = psum.tile([128, QC], f32, name="rope_swp", tag="mm")
                        nc.tensor.matmul(sw_ps[:, :n], perm[:], raw[:, :n],
                                         start=True, stop=True)
                        t1 = scr.tile([128, QC], bf16, name="rope_t1")
                        nc.vector.tensor_mul(out=t1[:, :n], in0=raw[:, :n],
                                             in1=ctb[:, c0:c1])
                        t2 = scr.tile([128, QC], bf16, name="rope_t2")
                        nc.vector.tensor_mul(out=t2[:, :n], in0=sw_ps[:, :n],
                                             in1=stb[:, c0:c1])
                        nc.vector.tensor_add(out=dest[:, mt, c0:c1],
                                             in0=t1[:, :n], in1=t2[:, :n])

            def attn_norm_store(po, dest_slice, n):
                """dest = po[:HD]/po[HD] columnwise (softmax denominator)."""
                rs = scr.tile([1, QC], f32, name="attn_rs")
                nc.vector.reciprocal(out=rs[:, :n], in_=po[HD:HD + 1, :n])
                rb_sb = scr.tile([64, QC], f32, name="attn_rb_sb")
                nc.gpsimd.partition_broadcast(rb_sb[:, :n], rs[:, :n])
                nc.vector.tensor_mul(out=dest_slice, in0=po[:HD, :n],
                                     in1=rb_sb[:, :n])

            def attention_full():
                for hh in range(NH):
                    tq = hh // 2
                    rq = slice(64 * (hh % 2), 64 * (hh % 2) + 64)
                    p_sb = ppool.tile([128, TT, NTOK], bf16, name="p_sb")
                    for mt in range(TT):
                        mr = _tok_rows(mt)
                        for ch in range(2):
                            if mt not in CH_MTS[ch]:
                                continue
                            cs = slice(ch * QC, (ch + 1) * QC)
                            ps = psum.tile([128, QC], f32, name="score_ps", tag="mm")
                            nc.tensor.matmul(
                                ps[:mr, :],
                                kTt[rq, tq, mt * 128:mt * 128 + mr],
                                qT[rq, tq, cs], start=True, stop=True)
                            nc.scalar.activation(
                                out=p_sb[:mr, mt, cs], in_=ps[:mr, :],
                                func=mybir.ActivationFunctionType.Exp,
                                scale=1.0 / math.sqrt(HD))
                        m0, m1 = MASK_SLICES[mt]
                        nc.vector.tensor_mul(
                            out=p_sb[:mr, mt, m0:m1], in0=p_sb[:mr, mt, m0:m1],
                            in1=mask[:mr, mt, m0:m1])
                    for ch in range(2):
                        cs = slice(ch * QC, (ch + 1) * QC)
                        mts = CH_MTS[ch]
                        po = psum.tile([128, QC], f32, name="pv_ps", tag="mm")
                        for i, mt in enumerate(mts):
                            mr = _tok_rows(mt)
                            nc.tensor.matmul(
                                po[:HD + 1, :],
                                v_aug[:mr, mt, hh * (HD + 1):(hh + 1) * (HD + 1)],
                                p_sb[:mr, mt, cs],
                                start=(i == 0), stop=(i == len(mts) - 1))
                        attn_norm_store(po, oT[rq, tq, cs], QC)

            def attention_thin_unbatched():
                for hh in range(NH):
                    tq = hh // 2
                    rq = slice(64 * (hh % 2), 64 * (hh % 2) + 64)
                    p_sb = ppool.tile([128, TT, NQ], bf16, name="pq_sb")
                    for mt in range(TT):
                        mr = _tok_rows(mt)
                        ps = psum.tile([128, QC], f32, name="score_ps", tag="mm")
                        nc.tensor.matmul(
                            ps[:mr, :NQ],
                            kTt[rq, tq, mt * 128:mt * 128 + mr],
                            qTq[rq, tq, :], start=True, stop=True)
                        nc.scalar.activation(
                            out=p_sb[:mr, mt, :], in_=ps[:mr, :NQ],
                            func=mybir.ActivationFunctionType.Exp,
                            scale=1.0 / math.sqrt(HD))
                        nc.vector.tensor_mul(
                            out=p_sb[:mr, mt, :], in0=p_sb[:mr, mt, :],
                            in1=mq[:mr, mt, 0:NQ])
                    po = psum.tile([128, QC], f32, name="pv_ps", tag="mm")
                    for mt in range(TT):
                        mr = _tok_rows(mt)
                        nc.tensor.matmul(
                            po[:HD + 1, :NQ],
                            v_aug[:mr, mt, hh * (HD + 1):(hh + 1) * (HD + 1)],
                            p_sb[:mr, mt, :],
                            start=(mt == 0), stop=(mt == TT - 1))
                    attn_norm_store(po, oTq[rq, tq, :], NQ)

            def attention_thin_batched():
                """All 8 heads batched side-by-side: scores/exp/mask/PV in
                [*, 8*NQ] tiles to amortize per-op overhead."""
                HB = NH * NQ  # 128
                p_sb = ppool.tile([128, TT, HB], bf16, name="pq_sb")
                for mt in range(TT):
                    mr = _tok_rows(mt)
                    ps = psum.tile([128, HB], f32, name="score_ps", tag="mm")
                    for hh in range(NH):
                        tq = hh // 2
                        rq = slice(64 * (hh % 2), 64 * (hh % 2) + 64)
                        nc.tensor.matmul(
                            ps[:mr, hh * NQ:(hh + 1) * NQ],
                            kTt[rq, tq, mt * 128:mt * 128 + mr],
                            qTq[rq, tq, :], start=True, stop=True)
                    nc.scalar.activation(
                        out=p_sb[:mr, mt, :], in_=ps[:mr, :],
                        func=mybir.ActivationFunctionType.Exp,
                        scale=1.0 / math.sqrt(HD))
                    nc.vector.tensor_mul(
                        out=p_sb[:mr, mt, :], in0=p_sb[:mr, mt, :],
                        in1=mq[:mr, mt, :])
                po = psum.tile([128, HB], f32, name="pv_ps", tag="mm")
                for hh in range(NH):
                    for mt in range(TT):
                        mr = _tok_rows(mt)
                        nc.tensor.matmul(
                            po[:HD + 1, hh * NQ:(hh + 1) * NQ],
                            v_aug[:mr, mt, hh * (HD + 1):(hh + 1) * (HD + 1)],
                            p_sb[:mr, mt, hh * NQ:(hh + 1) * NQ],
                            start=(mt == 0), stop=(mt == TT - 1))
                rs = scr.tile([1, HB], f32, name="attn_rsq")
                nc.vector.reciprocal(out=rs[:], in_=po[HD:HD + 1, :])
                rb_sb = scr.tile([64, HB], f32, name="attn_rbq")
                nc.gpsimd.partition_broadcast(rb_sb[:], rs[:])
                oq = scr.tile([64, HB], bf16, name="oq_flat")
                nc.vector.tensor_mul(out=oq[:], in0=po[:HD, :], in1=rb_sb[:])
                for hh in range(NH):
                    nc.gpsimd.tensor_copy(
                        out=oTq[64 * (hh % 2):64 * (hh % 2) + 64, hh // 2, :],
                        in_=oq[:, hh * NQ:(hh + 1) * NQ])

            def accum_proj(w_sb, src, n_k_tiles, dest, chunks):
                """dest += (src.T @ W).T via lhsT=W[kt,:], rhs=src[kt]."""
                for mt in range(KT):
                    for c0, c1 in chunks:
                        n = c1 - c0
                        ps = psum.tile([128, QC], f32, name="acc_ps", tag="mm")
                        for kt in range(n_k_tiles):
                            nc.tensor.matmul(
                                ps[:, :n], w_sb[:, kt, mt * 128:(mt + 1) * 128],
                                src[:, kt, c0:c1],
                                start=(kt == 0), stop=(kt == n_k_tiles - 1))
                        nc.vector.tensor_add(out=dest[:, mt, c0:c1],
                                             in0=dest[:, mt, c0:c1], in1=ps[:, :n])

            def mlp(xn, gdest, chunks, w1_sb, w3_sb, w2_sb, dest):
                for mt in range(HT):
                    for c0, c1 in chunks:
                        n = c1 - c0
                        ps3 = psum.tile([128, QC], f32, name="g3_ps", tag="mm")
                        for kt in range(KT):
                            nc.tensor.matmul(
                                ps3[:, :n], w3_sb[:, kt, mt * 128:(mt + 1) * 128],
                                xn[:, kt, c0:c1],
                                start=(kt == 0), stop=(kt == KT - 1))
                        g3c = scr.tile([128, QC], bf16, name="g3c")
                        nc.vector.tensor_copy(out=g3c[:, :n], in_=ps3[:, :n])
                        ps1 = psum.tile([128, QC], f32, name="g1_ps", tag="mm")
                        for kt in range(KT):
                            nc.tensor.matmul(
                                ps1[:, :n], w1_sb[:, kt, mt * 128:(mt + 1) * 128],
                                xn[:, kt, c0:c1],
                                start=(kt == 0), stop=(kt == KT - 1))
                        nc.scalar.activation(
                            out=gdest[:, mt, c0:c1], in_=ps1[:, :n],
                            func=mybir.ActivationFunctionType.Silu)
                        nc.vector.tensor_mul(
                            out=gdest[:, mt, c0:c1], in0=gdest[:, mt, c0:c1],
                            in1=g3c[:, :n])
                accum_proj(w2_sb, gdest, HT, dest, chunks)

            def mlp_thin(xn, gdest, w1_sb, w3_sb, w2_sb, dest):
                """All HT hidden tiles batched into [128, HT*NQ] psums."""
                HB = HT * NQ  # 176
                gflat = gdest.rearrange("p h q -> p (h q)")
                ps3 = psum.tile([128, HB], f32, name="g3_ps", tag="mm")
                for mt in range(HT):
                    for kt in range(KT):
                        nc.tensor.matmul(
                            ps3[:, mt * NQ:(mt + 1) * NQ],
                            w3_sb[:, kt, mt * 128:(mt + 1) * 128],
                            xn[:, kt, :],
                            start=(kt == 0), stop=(kt == KT - 1))
                g3c = scr.tile([128, HB], bf16, name="g3cq")
                nc.vector.tensor_copy(out=g3c[:], in_=ps3[:])
                ps1 = psum.tile([128, HB], f32, name="g1_ps", tag="mm")
                for mt in range(HT):
                    for kt in range(KT):
                        nc.tensor.matmul(
                            ps1[:, mt * NQ:(mt + 1) * NQ],
                            w1_sb[:, kt, mt * 128:(mt + 1) * 128],
                            xn[:, kt, :],
                            start=(kt == 0), stop=(kt == KT - 1))
                nc.scalar.activation(out=gflat[:], in_=ps1[:],
                                     func=mybir.ActivationFunctionType.Silu)
                nc.vector.tensor_mul(out=gflat[:], in0=gflat[:], in1=g3c[:])
                accum_proj(w2_sb, gdest, HT, dest, THIN_CH)

            def gather_q(dest, src):
                """dest[:, kt, 0]=src col 507; dest[:, kt, 1+3l+j]=src col 508+4l+j"""
                for kt in range(KT):
                    nc.vector.tensor_copy(out=dest[:, kt, 0:1],
                                          in_=src[:, kt, T - 1:T])
                    nc.vector.tensor_copy(
                        out=dest[:, kt, 1:NQ].rearrange("p (l s) -> p l s", s=3),
                        in_=src[:, kt, T:T + SUF].rearrange(
                            "p (l s) -> p l s", s=LBL)[:, :, 0:3])

            FULL_CH = ((0, QC), (QC, NTOK))
            THIN_CH = ((0, NQ),)

            # ================= transformer =================
            for l in range(NL):
                full = l < NL - 1
                wq_sb = wpool.tile([128, KT, D], bf16, name="wq_sb")
                wk_sb = wpool.tile([128, KT, D], bf16, name="wk_sb")
                wv_sb = wpool.tile([128, KT, D], bf16, name="wv_sb")
                wo_sb = wpool.tile([128, KT, D], bf16, name="wo_sb")
                w1_sb = wpool.tile([128, KT, HID], bf16, name="w1_sb")
                w3_sb = wpool.tile([128, KT, HID], bf16, name="w3_sb")
                w2_sb = wpool.tile([128, HT, D], bf16, name="w2_sb")
                for wsb, wd in ((wk_sb, wk), (wv_sb, wv), (wq_sb, wq),
                                (wo_sb, wo), (w1_sb, w1), (w3_sb, w3)):
                    nc.sync.dma_start(
                        out=wsb[:], in_=wd[l].rearrange("(k p) n -> p k n", p=128))
                nc.sync.dma_start(
                    out=w2_sb[:], in_=w2[l].rearrange("(k p) n -> p k n", p=128))

                xn = scr.tile([128, KT, NTOK], bf16, name="xn", bufs=1)
                rms(h, xn, NTOK, FULL_CH)
                # k/v always full (all tokens are keys); q right after k so
                # attention can begin before the v projection finishes
                proj_T(kTt, wk_sb, xn, FULL_CH, rope=True, ctb=C128, stb=S128)
                if l < NL - 1:
                    proj_T(qT, wq_sb, xn, FULL_CH, rope=True, ctb=C128, stb=S128)
                for mt in range(TT):
                    mr = _tok_rows(mt)
                    ps = psum.tile([128, D], f32, name="v_ps", tag="mm")
                    for kt in range(KT):
                        nc.tensor.matmul(
                            ps[:mr, :], xn[:, kt, mt * 128:mt * 128 + mr],
                            wv_sb[:, kt, :],
                            start=(kt == 0), stop=(kt == KT - 1))
                    nc.vector.tensor_copy(
                        out=v_aug.rearrange("p t (h c) -> p t h c", c=HD + 1)[
                            :mr, mt, :, :HD],
                        in_=ps.rearrange("p (h c) -> p h c", c=HD)[:mr, :, :])

                if full:
                    attention_full()
                    accum_proj(wo_sb, oT, KT, h, FULL_CH)
                    xn2 = scr.tile([128, KT, NTOK], bf16, name="xn", bufs=1)
                    rms(h, xn2, NTOK, FULL_CH)
                    mlp(xn2, g1, FULL_CH, w1_sb, w3_sb, w2_sb, h)
                else:
                    gather_q(hq, h)
                    xnq = scr.tile([128, KT, NQ], bf16, name="xnq")
                    gather_q(xnq, xn)
                    proj_T(qTq, wq_sb, xnq, THIN_CH, rope=True, ctb=Cq, stb=Sq)
                    if BATCH_THIN:
                        attention_thin_batched()
                    else:
                        attention_thin_unbatched()
                    accum_proj(wo_sb, oTq, KT, hq, THIN_CH)
                    xnq2 = scr.tile([128, KT, NQ], bf16, name="xnq2")
                    rms(hq, xnq2, NQ, THIN_CH)
                    if BATCH_THIN:
                        mlp_thin(xnq2, g1q, w1_sb, w3_sb, w2_sb, hq)
                    else:
                        mlp(xnq2, g1q, THIN_CH, w1_sb, w3_sb, w2_sb, hq)

            # ============ final norm + extract + AllGather ============
            xnf = scr.tile([128, KT, NQ], bf16, name="xnf")
            rms(hq, xnf, NQ, THIN_CH)
            hsT_own = acts.tile([128, KT, NSEL], bf16, name="hsT_own")
            for kt in range(KT):
                for ll in range(NLAB):
                    nc.scalar.copy(
                        out=hsT_own[:, kt, ll * LBL:ll * LBL + 1],
                        in_=xnf[:, kt, 0:1])
                nc.scalar.copy(
                    out=hsT_own.rearrange("p k (l s) -> p k l s", s=LBL)[
                        :, kt, :, 1:LBL],
                    in_=xnf[:, kt, 1:NQ].rearrange("p (l s) -> p l s", s=3))

            cc_in = dram.tile([D, NSEL], bf16)
            cc_out = dram.tile([NCORES * D, NSEL], bf16)
            nc.sync.dma_start(
                out=cc_in.rearrange("(k p) c -> p k c", p=128), in_=hsT_own[:])
            if use_collective:
                nc.gpsimd.collective_compute(
                    "AllGather",
                    mybir.AluOpType.bypass,
                    replica_groups=[list(range(NCORES))],
                    ins=[cc_in.opt()],
                    outs=[cc_out.opt()],
                )
            else:  # timeline-sim variant: emulate with local copies
                for r in range(NCORES):
                    nc.sync.dma_start(
                        out=cc_out[r * D:(r + 1) * D, :], in_=cc_in[:])

            # hsT_all[kt]: [128, B, NSEL] from ranks 0..3 of the gather
            hsT_all = acts.tile([128, KT, B, NSEL], bf16, name="hsT_all")
            cc_view = cc_out.rearrange("(b k p) c -> p k b c", b=NCORES, p=128)
            for kt in range(KT):
                nc.sync.dma_start(out=hsT_all[:, kt], in_=cc_view[:, kt, 0:B, :])

            # ================= lm_head phase =================
            lmsel_sb = consts.tile([128, KT, NSEL], bf16)
            nc.sync.dma_start(
                out=lmsel_sb[:], in_=lmsel.rearrange("(k p) c -> p k c", p=128))
            se_sb = acts.tile([NROW, NVCH], f32, name="se_sb")
            if USE_FP8:
                # fp8 DoubleRow copies of hs: [128, pass, 2, 80]
                hs8 = acts.tile([128, 2, 2, B * NSEL], fp8, name="hs8")
                for kt in range(KT):
                    nc.vector.tensor_scalar_mul(
                        out=hs8[:, kt // 2, kt % 2, :],
                        in0=hsT_all.rearrange("p k b c -> p k (b c)")[:, kt, :],
                        scalar1=HS_SCALE)
                lmh_v = lmh.rearrange("a p s v -> p a s v")
            for j in range(NVCH):
                pl = psum.tile([NROW, VCH], f32, name="lm_ps", tag="mm")
                if USE_FP8:
                    lsb = lpool.tile([128, 2, 2, VCH], fp8, name="lsb")
                    for pp in range(2):
                        nc.sync.dma_start(
                            out=lsb[:, pp],
                            in_=lmh_v[:, pp, :, j * VCH:(j + 1) * VCH])
                    for pp in range(2):
                        nc.tensor.matmul(pl[:], hs8[:, pp], lsb[:, pp],
                                         start=(pp == 0), stop=(pp == 1),
                                         perf_mode=mybir.MatmulPerfMode.DoubleRow)
                    escale = 1.0 / (LMH_SCALE * HS_SCALE)
                else:
                    lsb = lpool.tile([128, KT, VCH], bf16, name="lsb")
                    nc.sync.dma_start(
                        out=lsb[:],
                        in_=lmh.rearrange("(k p) v -> p k v", p=128)[
                            :, :, j * VCH:(j + 1) * VCH])
                    for kt in range(KT):
                        nc.tensor.matmul(pl[:], hsT_all[:, kt], lsb[:, kt, :],
                                         start=(kt == 0), stop=(kt == KT - 1))
                    escale = 1.0
                esc = scr.tile([NROW, VCH], f32, name="esc")
                nc.scalar.activation(
                    out=esc[:], in_=pl[:],
                    func=mybir.ActivationFunctionType.Exp,
                    scale=escale,
                    accum_out=se_sb[:, j:j + 1])
            nc.sync.dma_start(out=se_out[:], in_=se_sb[:])

            psel = psmall.tile([NROW, NSEL], f32, name="sel_ps", tag="small")
            for kt in range(KT):
                nc.tensor.matmul(psel[:], hsT_all[:, kt], lmsel_sb[:, kt, :],
                                 start=(kt == 0), stop=(kt == KT - 1))
            sel_sb = scr.tile([NROW, NSEL], f32, name="sel_sb")
            nc.scalar.copy(out=sel_sb[:], in_=psel[:])
            nc.sync.dma_start(out=sel_out[:], in_=sel_sb[:])

    nc.finalize()
    return nc


def _get_nc():
    if "nc" not in _CACHE:
        _CACHE["nc"] = build_nc()
    return _CACHE["nc"]


def _build_masks():
    """full mask [TT,128,NTOK] and thin mask [TT,128,NQ] over (k, q)."""
    k_idx = np.arange(TT * 128)
    kpos = np.where(k_idx < T, k_idx, 0)
    klab = np.where(k_idx < T, -1, (k_idx - T) // LBL)
    koff = np.where(k_idx < T, 0, (k_idx - T) % LBL)
    kvalid = k_idx < NTOK

    def allow(qpos, qlab, qoff):
        kp = kpos[:, None]; kl = klab[:, None]; ko = koff[:, None]
        prefix_k = kl == -1
        prefix_q = (qlab == -1)[None, :]
        a = np.where(
            prefix_q,
            prefix_k & (kp <= qpos[None, :]),
            prefix_k | ((kl == qlab[None, :]) & (ko <= qoff[None, :])),
        )
        return (a & kvalid[:, None]).astype(np.float32)

    q_idx = np.arange(NTOK)
    qpos = np.where(q_idx < T, q_idx, 0)
    qlab = np.where(q_idx < T, -1, (q_idx - T) // LBL)
    qoff = np.where(q_idx < T, 0, (q_idx - T) % LBL)
    maskd = allow(qpos, qlab, qoff).reshape(TT, 128, NTOK).astype(BF16)

    # thin queries: col 0 = token 507; col 1+3l+j = token 508+4l+j (j=0..2)
    tq = np.array([T - 1] + [T + 4 * l + j for l in range(NLAB) for j in range(3)])
    qpos = np.where(tq < T, tq, 0)
    qlab = np.where(tq < T, -1, (tq - T) // LBL)
    qoff = np.where(tq < T, 0, (tq - T) % LBL)
    mq1 = allow(qpos, qlab, qoff)                       # [TT*128, NQ]
    maskqa = np.tile(mq1, (1, NH)).reshape(TT, 128, NH * NQ).astype(BF16)
    return maskd, maskqa, tq


def _host_prep(inputs):
    """Build per-core in_maps from full inputs."""
    input_ids = np.asarray(inputs["input_ids"])
    label_ids = np.asarray(inputs["label_ids"])
    emb = np.asarray(inputs["emb"], dtype=np.float32)
    anw = np.asarray(inputs["attn_norm_w"], dtype=np.float32)
    fnw = np.asarray(inputs["ffn_norm_w"], dtype=np.float32)
    finw = np.asarray(inputs["final_norm_w"], dtype=np.float32)
    lm_head = np.asarray(inputs["lm_head"], dtype=np.float32)

    # fold norm weights into the consuming matmuls
    wq = np.asarray(inputs["wq"], np.float32) * anw[:, :, None]
    wk = np.asarray(inputs["wk"], np.float32) * anw[:, :, None]
    wv = np.asarray(inputs["wv"], np.float32) * anw[:, :, None]
    wo = np.asarray(inputs["wo"], np.float32)
    w1 = np.asarray(inputs["w1"], np.float32) * fnw[:, :, None]
    w3 = np.asarray(inputs["w3"], np.float32) * fnw[:, :, None]
    w2 = np.asarray(inputs["w2"], np.float32)
    lmh_f = lm_head * finw[:, None]

    suf_ids = label_ids.reshape(-1)  # (l, j) order

    # RoPE tables: packed col -> position
    pos = np.concatenate(
        [np.arange(T), np.tile(T + np.arange(LBL), NLAB)]).astype(np.float32)
    freqs = 1.0 / (10000.0 ** (np.arange(HALF, dtype=np.float32) / HALF))

    def rope_tabs(positions):
        ang = positions[None, :] * freqs[:, None]
        c = np.tile(np.cos(ang), (4, 1)).astype(BF16)
        s32 = np.sin(ang)
        s = np.concatenate([-s32, s32, -s32, s32], 0).astype(BF16)
        return c, s

    ctab, stab = rope_tabs(pos)
    maskd, maskqa, tq = _build_masks()
    cqt, sqt = rope_tabs(pos[tq])

    sigma = np.arange(128)
    sigma = (sigma // 64) * 64 + ((sigma % 64 + 32) % 64)
    permm = np.zeros((128, 128), dtype=np.float32)
    permm[sigma, np.arange(128)] = 1.0
    permm = permm.astype(BF16)

    sel_cols = suf_ids.astype(np.int64)
    lmsel = np.ascontiguousarray(lmh_f[:, sel_cols]).astype(BF16)

    common = dict(
        wq=wq.astype(BF16), wk=wk.astype(BF16), wv=wv.astype(BF16),
        wo=wo.astype(BF16), w1=w1.astype(BF16), w3=w3.astype(BF16),
        w2=w2.astype(BF16), ctab=ctab, stab=stab, cqtab=cqt, sqtab=sqt,
        maskd=maskd, maskq=maskqa, lmsel=lmsel, permd=permm,
    )
    if USE_FP8:
        # fp8 DoubleRow layout: [pass, 128, 2, V] with K row (a*2+s)*128+p
        FP8 = np.dtype(ml_dtypes.float8_e4m3)
        lmh8 = (lmh_f * LMH_SCALE).astype(FP8).reshape(2, 2, 128, V)
        lmh8 = np.ascontiguousarray(lmh8.transpose(0, 2, 1, 3))
        shards = [np.ascontiguousarray(lmh8[:, :, :, c * VSH:(c + 1) * VSH])
                  for c in range(NCORES)]
    else:
        lmh_bf = lmh_f.astype(BF16)
        shards = [np.ascontiguousarray(lmh_bf[:, c * VSH:(c + 1) * VSH])
                  for c in range(NCORES)]

    in_maps = []
    for c in range(NCORES):
        b = c % B
        tok = np.concatenate([input_ids[b], suf_ids])
        x0 = emb[tok]                      # [528, 512] fp32
        m = dict(common)
        m["x0T"] = np.ascontiguousarray(x0.T)
        m["lmh"] = shards[c]
        in_maps.append(m)
    return in_maps


def _host_combine(results):
    """Combine per-core partial sumexp + selected logits into [B, NLAB]."""
    se = np.zeros((NROW,), dtype=np.float64)
    for c in range(NCORES):
        se += np.asarray(results[c]["se_out"], np.float64).sum(axis=1)
    lse = np.log(se)  # [80], rows ordered (b, l, j)
    sel = np.asarray(results[0]["sel_out"], np.float64)  # [80, 20]
    rows = np.arange(NROW)
    bb = rows // (NLAB * LBL)
    ll = (rows % (NLAB * LBL)) // LBL
    jj = rows % LBL
    lp = sel[rows, ll * LBL + jj] - lse  # [80]
    out = np.zeros((B, NLAB), dtype=np.float64)
    np.add.at(out, (bb, ll), lp)
    return out.astype(np.float32)


def kernel(**inputs):
    nc = _get_nc()
    in_maps = _host_prep(inputs)
    res = run_bass_kernel_spmd(
        nc, in_maps, core_ids=list(range(NCORES)),
        trace=_CACHE.get("trace", False),
    )
    _CACHE["last_results"] = res
    return _host_combine(res.results)



# revision 35
# speedup vs baseline: 1.0139x; 1.0139x over previous
"""Trainium2 Bass kernel for nn_LlamaEmbeddingClassifier.

Model: 2-layer Llama (D=512, 8 heads x 64, HID=1408, RoPE, RMSNorm) scoring
B=4 prompts against NLAB=5 label continuations (LBL=4 tokens) with an
lm_head over V=128000.

Strategy (8 NeuronCores, single SPMD launch):
  - The 5 label variants of each batch row share a 508-token prefix; causal
    attention means prefix activations are label-independent. Each core runs
    the transformer over a packed 528-token sequence
    [508 prefix | 5 x 4-token suffixes] with a custom attention mask.
    Core c handles batch row (c % 4); cores 4-7 duplicate 0-3 (the SPMD
    program is uniform; their phase-1 output is ignored).
  - Only 16 positions feed the classifier (pos 507 + first 3 tokens of each
    suffix), so layer 2 runs full K/V projections but a THIN query path
    (16 queries) for attention/MLP — layer-2 cost collapses to ~K/V cost.
  - Activations live TRANSPOSED in SBUF as [D(partitions), tokens] so every
    projection uses the weight matrices as-given for lhsT and never needs an
    on-device transpose. Scores are computed transposed [k, q]; softmax is
    un-normalized exp (scores are bounded ~|1.5|, so no max-subtraction),
    with the row-sum obtained free via a ones-column appended to V.
    Fully-masked causal blocks are skipped.
  - The 16 final hidden rows per batch are AllGathered (tiny) so every core
    holds all 80 scoring rows.
  - lm_head [512,128000] is the memory bottleneck: vocab-sharded 8 ways,
    streamed as scaled fp8-e4m3 in DoubleRow perf mode (halves both HBM
    traffic and PE cycles; the fp8 noise averages out in the logsumexp);
    each core computes partial sumexp over its 16000 columns.  Label-token
    logits come from a tiny bf16 side matmul against the 20 selected
    lm_head columns, so the accuracy-critical path avoids fp8.  Host
    combines: logsumexp across shards, lp = sel_logit - lse, summed per
    (batch,label).
  - Other matmuls bf16 with fp32 PSUM accumulation; norms/softmax sums fp32.
"""

import math
import os
import sys
from contextlib import ExitStack

for _p in ("/opt/trn_rl_repo", "/root/.axon_site/_ro/trn_rl_repo"):
    if os.path.isdir(_p) and _p not in sys.path:
        sys.path.insert(0, _p)

import ml_dtypes
import numpy as np

import concourse.bass as bass
import concourse.tile as tile
from concourse import bacc, mybir
from concourse.bass_utils import run_bass_kernel_spmd

BF16 = ml_dtypes.bfloat16

# Problem dims (hardcoded per contract)
V, D, NH, NL, HID = 128000, 512, 8, 2, 1408
HD, HALF = 64, 32
B, T, NLAB, LBL = 4, 508, 5, 4
EPS = 1e-5
NCORES = 8
SUF = NLAB * LBL            # 20 suffix tokens
NTOK = T + SUF              # 528 packed tokens
KT = D // 128               # 4 K-tiles over D
TT = (NTOK + 127) // 128    # 5 token tiles (last has 16 rows)
HT = HID // 128             # 11 tiles over HID
NROW = B * NLAB * LBL       # 80 scoring rows
NSEL = NLAB * LBL           # 20 selected lm_head columns
NQ = 1 + NLAB * (LBL - 1)   # 16 thin-path query positions
VSH = V // NCORES           # 16000 vocab shard per core
VCH = 500                   # vocab chunk per matmul sweep
NVCH = VSH // VCH           # 32 chunks
LVCH = 500                  # lm_head chunk (psum bank limit: 500 f32 cols)
LNVCH = VSH // LVCH         # 32 chunks
QC = 264                    # q chunk (2 chunks of 264 = 528)
LMH_SCALE = 32.0            # host premultiply of fp8 lm_head (fp8 dynamic range)
HS_SCALE = 4.0              # device premultiply of fp8 hs copies

# causal block structure for the full layer: q-chunk 0 (q<264) only sees
# k-tiles 0,1; q-chunk 1 sees all.  Mask-multiply only where partial.
CH_MTS = {0: (0, 1), 1: (0, 1, 2, 3, 4)}
# mask only the partial blocks: ch0 triangles for mt 0,1; ch1 edges for mt>=2
MASK_SLICES = {0: (0, QC), 1: (0, QC),
               2: (QC, NTOK), 3: (QC, NTOK), 4: (QC, NTOK)}

_CACHE = {}
# NB: the "batched" thin path (multiple matmuls writing one PSUM tile at
# different free offsets) crashes real HW (NRT_EXEC_UNIT_UNRECOVERABLE) even
# though CoreSim/walrus accept it -- matmul PSUM outputs must start at the
# tile's offset 0.  Keep unbatched.
BATCH_THIN = os.environ.get("K_BATCH_THIN", "0") == "1"
USE_FP8 = os.environ.get("K_FP8", "1") == "1"
# residual stream dtype: bf16 halves x0 DMA + enables DVE 2x fast modes
H_BF16 = os.environ.get("K_HBF", "1") == "1"


def _tok_rows(tt):
    return min(128, NTOK - tt * 128)


def build_nc(use_collective=True):
    nc = bacc.Bacc("TRN2", num_devices=NCORES)
    f32, bf16 = mybir.dt.float32, mybir.dt.bfloat16

    hdt = bf16 if H_BF16 else f32

    # ---- I/O ----
    x0T = nc.dram_tensor("x0T", [D, NTOK], hdt, kind="ExternalInput")
    wq = nc.dram_tensor("wq", [NL, D, D], bf16, kind="ExternalInput")
    wk = nc.dram_tensor("wk", [NL, D, D], bf16, kind="ExternalInput")
    wv = nc.dram_tensor("wv", [NL, D, D], bf16, kind="ExternalInput")
    wo = nc.dram_tensor("wo", [NL, D, D], bf16, kind="ExternalInput")
    w1 = nc.dram_tensor("w1", [NL, D, HID], bf16, kind="ExternalInput")
    w3 = nc.dram_tensor("w3", [NL, D, HID], bf16, kind="ExternalInput")
    w2 = nc.dram_tensor("w2", [NL, HID, D], bf16, kind="ExternalInput")
    ctab = nc.dram_tensor("ctab", [128, NTOK], bf16, kind="ExternalInput")
    stab = nc.dram_tensor("stab", [128, NTOK], bf16, kind="ExternalInput")
    cqtab = nc.dram_tensor("cqtab", [128, NQ], bf16, kind="ExternalInput")
    sqtab = nc.dram_tensor("sqtab", [128, NQ], bf16, kind="ExternalInput")
    permd = nc.dram_tensor("permd", [128, 128], bf16, kind="ExternalInput")
    maskd = nc.dram_tensor("maskd", [TT, 128, NTOK], bf16, kind="ExternalInput")
    maskq = nc.dram_tensor("maskq", [TT, 128, NH * NQ], bf16, kind="ExternalInput")
    # lm_head shard in fp8 (DoubleRow layout: [pass, 128, 2, V]), scaled by
    # LMH_SCALE on host; compensated in the exp scale.
    fp8 = mybir.dt.float8e4
    if USE_FP8:
        lmh = nc.dram_tensor("lmh", [2, 128, 2, VSH], fp8, kind="ExternalInput")
    else:
        lmh = nc.dram_tensor("lmh", [D, VSH], bf16, kind="ExternalInput")
    lmsel = nc.dram_tensor("lmsel", [D, NSEL], bf16, kind="ExternalInput")

    se_out = nc.dram_tensor("se_out", [NROW, LNVCH], f32, kind="ExternalOutput")
    sel_out = nc.dram_tensor("sel_out", [NROW, NSEL], f32, kind="ExternalOutput")

    with tile.TileContext(nc) as tc, ExitStack() as ctx:
        consts = ctx.enter_context(tc.tile_pool(name="consts", bufs=1))
        wpool = ctx.enter_context(tc.tile_pool(name="weights", bufs=1))
        acts = ctx.enter_context(tc.tile_pool(name="acts", bufs=1))
        scr = ctx.enter_context(tc.tile_pool(name="scratch", bufs=3))
        ppool = ctx.enter_context(tc.tile_pool(name="p", bufs=2))
        psum = ctx.enter_context(tc.tile_pool(name="psum", bufs=6, space="PSUM"))
        psmall = ctx.enter_context(tc.tile_pool(name="psmall", bufs=2, space="PSUM"))
        lpool = ctx.enter_context(tc.tile_pool(name="lmh", bufs=20))
        dram = ctx.enter_context(tc.tile_pool(name="dram", bufs=1, space="DRAM"))
        if True:
            # ---- constants ----
            ones_col = consts.tile([128, 1], bf16)       # rms partition-reduce lhsT
            nc.vector.memset(ones_col, 1.0)
            eps_sb = consts.tile([1, 1], f32)
            nc.vector.memset(eps_sb, float(EPS))
            C128 = consts.tile([128, NTOK], bf16)
            S128 = consts.tile([128, NTOK], bf16)
            nc.sync.dma_start(out=C128[:], in_=ctab[:])
            nc.sync.dma_start(out=S128[:], in_=stab[:])
            Cq = consts.tile([128, NQ], bf16)
            Sq = consts.tile([128, NQ], bf16)
            nc.sync.dma_start(out=Cq[:], in_=cqtab[:])
            nc.sync.dma_start(out=Sq[:], in_=sqtab[:])
            perm = consts.tile([128, 128], bf16)
            nc.sync.dma_start(out=perm[:], in_=permd[:])
            # residual stream hT [D, tok] -- loaded first (first consumer)
            h = acts.tile([128, KT, NTOK], hdt)
            nc.sync.dma_start(out=h[:], in_=x0T.rearrange("(k p) q -> p k q", p=128))
            mask = consts.tile([128, TT, NTOK], bf16)
            nc.sync.dma_start(out=mask[:], in_=maskd.rearrange("t p q -> p t q"))
            mq = consts.tile([128, TT, NH * NQ], bf16)
            nc.sync.dma_start(out=mq[:], in_=maskq.rearrange("t p q -> p t q"))

            # v with interleaved ones column: [tok, 8 heads x (64 v | 1 one)]
            v_aug = acts.tile([128, TT, NH * (HD + 1)], bf16)
            nc.vector.memset(
                v_aug.rearrange("p t (h c) -> p t h c", c=HD + 1)[:, :, :, HD], 1.0
            )

            if USE_FP8:
                # full vocab-shard of lm_head, resident in SBUF
                # [128, pass, 2, VSH] fp8 -- 64KB/partition
                lsb_all = acts.tile([128, 2, 2, VSH], fp8, name="lsb_all")

            qT = acts.tile([128, KT, NTOK], bf16, name="qT")
            kTt = acts.tile([128, KT, NTOK], bf16, name="kTt")
            oT = acts.tile([128, KT, NTOK], bf16, name="oT")
            g1 = acts.tile([128, HT, NTOK], bf16, name="g1")
            hq = acts.tile([128, KT, NQ], f32, name="hq")
            qTq = acts.tile([128, KT, NQ], bf16, name="qTq")
            oTq = acts.tile([128, KT, NQ], bf16, name="oTq")
            g1q = acts.tile([128, HT, NQ], bf16, name="g1q")

            def rms(src, dest, n, chunks):
                """dest (bf16) = src * rsqrt(mean_D(src^2)+eps); [128,KT,n]"""
                sq = scr.tile([128, KT, n], bf16, name=f"sq{n}", bufs=1)
                for kt in range(KT):
                    nc.scalar.activation(out=sq[:, kt, :], in_=src[:, kt, :],
                                         func=mybir.ActivationFunctionType.Square)
                rstd = scr.tile([1, n], f32, name=f"rstd{n}")
                for c0, c1 in chunks:
                    ss = psmall.tile([1, c1 - c0], f32, name="ss", tag="small")
                    for kt in range(KT):
                        nc.tensor.matmul(ss[:], ones_col[:], sq[:, kt, c0:c1],
                                         start=(kt == 0), stop=(kt == KT - 1))
                    nc.scalar.activation(out=rstd[:, c0:c1], in_=ss[:],
                                         func=mybir.ActivationFunctionType.Sqrt,
                                         scale=1.0 / D, bias=eps_sb[:])
                nc.vector.reciprocal(out=rstd[:], in_=rstd[:])
                if H_BF16:
                    rstd_h = scr.tile([1, n], hdt, name=f"rstdh{n}")
                    nc.vector.tensor_copy(out=rstd_h[:], in_=rstd[:])
                    rstd = rstd_h
                rb = scr.tile([128, n], hdt, name=f"rms_rb{n}")
                nc.gpsimd.partition_broadcast(rb[:], rstd[:])
                for kt in range(KT):
                    nc.vector.tensor_mul(out=dest[:, kt, :], in0=src[:, kt, :],
                                         in1=rb[:])

            def proj_T(dest, w_sb, xn, chunks, rope, ctb=None, stb=None):
                """dest[Dout, n] = (xn @ W).T via lhsT=W; optional RoPE."""
                for mt in range(KT):
                    for c0, c1 in chunks:
                        n = c1 - c0
                        ps = psum.tile([128, QC], f32, name="proj_ps", tag="mm")
                        for kt in range(KT):
                            nc.tensor.matmul(
                                ps[:, :n], w_sb[:, kt, mt * 128:(mt + 1) * 128],
                                xn[:, kt, c0:c1],
                                start=(kt == 0), stop=(kt == KT - 1))
                        if not rope:
                            nc.scalar.copy(out=dest[:, mt, c0:c1], in_=ps[:, :n])
                            continue
                        # RoPE: out = raw*C + (perm @ raw)*S
                        raw = scr.tile([128, QC], bf16, name="rope_raw")
                        nc.vector.tensor_copy(out=raw[:, :n], in_=ps[:, :n])
                        sw_ps = psum.tile([128, QC], f32, name="rope_swp", tag="mm")
                        nc.tensor.matmul(sw_ps[:, :n], perm[:], raw[:, :n],
                                         start=True, stop=True)
                        t1 = scr.tile([128, QC], bf16, name="rope_t1")
                        nc.vector.tensor_mul(out=t1[:, :n], in0=raw[:, :n],
                                             in1=ctb[:, c0:c1])
                        t2 = scr.tile([128, QC], bf16, name="rope_t2")
                        nc.vector.tensor_mul(out=t2[:, :n], in0=sw_ps[:, :n],
                                             in1=stb[:, c0:c1])
                        nc.vector.tensor_add(out=dest[:, mt, c0:c1],
                                             in0=t1[:, :n], in1=t2[:, :n])

            def attn_norm_store(po, dest_slice, n):
                """dest = po[:HD]/po[HD] columnwise (softmax denominator)."""
                rs = scr.tile([1, QC], f32, name="attn_rs")
                nc.vector.reciprocal(out=rs[:, :n], in_=po[HD:HD + 1, :n])
                rb_sb = scr.tile([64, QC], f32, name="attn_rb_sb")
                nc.gpsimd.partition_broadcast(rb_sb[:, :n], rs[:, :n])
                nc.vector.tensor_mul(out=dest_slice, in0=po[:HD, :n],
                                     in1=rb_sb[:, :n])

            def attention_full():
                for hh in range(NH):
                    tq = hh // 2
                    rq = slice(64 * (hh % 2), 64 * (hh % 2) + 64)
                    p_sb = ppool.tile([128, TT, NTOK], bf16, name="p_sb")
                    for mt in range(TT):
                        mr = _tok_rows(mt)
                        for ch in range(2):
                            if mt not in CH_MTS[ch]:
                                continue
                            cs = slice(ch * QC, (ch + 1) * QC)
                            ps = psum.tile([128, QC], f32, name="score_ps", tag="mm")
                            nc.tensor.matmul(
                                ps[:mr, :],
                                kTt[rq, tq, mt * 128:mt * 128 + mr],
                                qT[rq, tq, cs], start=True, stop=True)
                            nc.scalar.activation(
                                out=p_sb[:mr, mt, cs], in_=ps[:mr, :],
                                func=mybir.ActivationFunctionType.Exp,
                                scale=1.0 / math.sqrt(HD))
                        m0, m1 = MASK_SLICES[mt]
                        nc.vector.tensor_mul(
                            out=p_sb[:mr, mt, m0:m1], in0=p_sb[:mr, mt, m0:m1],
                            in1=mask[:mr, mt, m0:m1])
                    for ch in range(2):
                        cs = slice(ch * QC, (ch + 1) * QC)
                        mts = CH_MTS[ch]
                        po = psum.tile([128, QC], f32, name="pv_ps", tag="mm")
                        for i, mt in enumerate(mts):
                            mr = _tok_rows(mt)
                            nc.tensor.matmul(
                                po[:HD + 1, :],
                                v_aug[:mr, mt, hh * (HD + 1):(hh + 1) * (HD + 1)],
                                p_sb[:mr, mt, cs],
                                start=(i == 0), stop=(i == len(mts) - 1))
                        attn_norm_store(po, oT[rq, tq, cs], QC)

            def attention_thin_unbatched():
                # block-diagonal q per head pair: one score matmul (K=128)
                # covers both heads of a tq tile; exp/mask run on [mr, 2*NQ].
                qblk = scr.tile([128, KT, 2 * NQ], bf16, name="qblk", bufs=1)
                nc.vector.memset(qblk[:], 0.0)
                for tq in range(KT):
                    nc.vector.tensor_copy(out=qblk[0:64, tq, 0:NQ],
                                          in_=qTq[0:64, tq, :])
                    nc.vector.tensor_copy(out=qblk[64:128, tq, NQ:2 * NQ],
                                          in_=qTq[64:128, tq, :])
                p_sb = acts.tile([128, TT, NH, NQ], bf16, name="pq_sb")
                for tq in range(KT):
                    for mt in range(TT):
                        mr = _tok_rows(mt)
                        ps = psum.tile([128, 2 * NQ], f32, name="score_ps",
                                       tag="mm")
                        nc.tensor.matmul(
                            ps[:mr, :],
                            kTt[:, tq, mt * 128:mt * 128 + mr],
                            qblk[:, tq, :], start=True, stop=True)
                        nc.scalar.activation(
                            out=p_sb[:mr, mt, 2 * tq:2 * tq + 2, :],
                            in_=ps[:mr, :],
                            func=mybir.ActivationFunctionType.Exp,
                            scale=1.0 / math.sqrt(HD))
                        nc.gpsimd.tensor_mul(
                            out=p_sb[:mr, mt, 2 * tq:2 * tq + 2, :],
                            in0=p_sb[:mr, mt, 2 * tq:2 * tq + 2, :],
                            in1=mq[:mr, mt, 2 * tq * NQ:(2 * tq + 2) * NQ])
                for hh in range(NH):
                    tq = hh // 2
                    rq = slice(64 * (hh % 2), 64 * (hh % 2) + 64)
                    po = psum.tile([128, QC], f32, name="pv_ps", tag="mm")
                    for mt in range(TT):
                        mr = _tok_rows(mt)
                        nc.tensor.matmul(
                            po[:HD + 1, :NQ],
                            v_aug[:mr, mt, hh * (HD + 1):(hh + 1) * (HD + 1)],
                            p_sb[:mr, mt, hh, :],
                            start=(mt == 0), stop=(mt == TT - 1))
                    attn_norm_store(po, oTq[rq, tq, :], NQ)

            def attention_thin_batched():
                """All 8 heads batched side-by-side: scores/exp/mask/PV in
                [*, 8*NQ] tiles to amortize per-op overhead."""
                HB = NH * NQ  # 128
                p_sb = ppool.tile([128, TT, HB], bf16, name="pq_sb")
                for mt in range(TT):
                    mr = _tok_rows(mt)
                    ps = psum.tile([128, HB], f32, name="score_ps", tag="mm")
                    for hh in range(NH):
                        tq = hh // 2
                        rq = slice(64 * (hh % 2), 64 * (hh % 2) + 64)
                        nc.tensor.matmul(
                            ps[:mr, hh * NQ:(hh + 1) * NQ],
                            kTt[rq, tq, mt * 128:mt * 128 + mr],
                            qTq[rq, tq, :], start=True, stop=True)
                    nc.scalar.activation(
                        out=p_sb[:mr, mt, :], in_=ps[:mr, :],
                        func=mybir.ActivationFunctionType.Exp,
                        scale=1.0 / math.sqrt(HD))
                    nc.vector.tensor_mul(
                        out=p_sb[:mr, mt, :], in0=p_sb[:mr, mt, :],
                        in1=mq[:mr, mt, :])
                po = psum.tile([128, HB], f32, name="pv_ps", tag="mm")
                for hh in range(NH):
                    for mt in range(TT):
                        mr = _tok_rows(mt)
                        nc.tensor.matmul(
                            po[:HD + 1, hh * NQ:(hh + 1) * NQ],
                            v_aug[:mr, mt, hh * (HD + 1):(hh + 1) * (HD + 1)],
                            p_sb[:mr, mt, hh * NQ:(hh + 1) * NQ],
                            start=(mt == 0), stop=(mt == TT - 1))
                rs = scr.tile([1, HB], f32, name="attn_rsq")
                nc.vector.reciprocal(out=rs[:], in_=po[HD:HD + 1, :])
                rb_sb = scr.tile([64, HB], f32, name="attn_rbq")
                nc.gpsimd.partition_broadcast(rb_sb[:], rs[:])
                oq = scr.tile([64, HB], bf16, name="oq_flat")
                nc.vector.tensor_mul(out=oq[:], in0=po[:HD, :], in1=rb_sb[:])
                for hh in range(NH):
                    nc.gpsimd.tensor_copy(
                        out=oTq[64 * (hh % 2):64 * (hh % 2) + 64, hh // 2, :],
                        in_=oq[:, hh * NQ:(hh + 1) * NQ])

            def accum_proj(w_sb, src, n_k_tiles, dest, chunks):
                """dest += (src.T @ W).T via lhsT=W[kt,:], rhs=src[kt]."""
                for mt in range(KT):
                    for c0, c1 in chunks:
                        n = c1 - c0
                        ps = psum.tile([128, QC], f32, name="acc_ps", tag="mm")
                        for kt in range(n_k_tiles):
                            nc.tensor.matmul(
                                ps[:, :n], w_sb[:, kt, mt * 128:(mt + 1) * 128],
                                src[:, kt, c0:c1],
                                start=(kt == 0), stop=(kt == n_k_tiles - 1))
                        nc.vector.tensor_add(out=dest[:, mt, c0:c1],
                                             in0=dest[:, mt, c0:c1], in1=ps[:, :n])

            def mlp(xn, gdest, chunks, w1_sb, w3_sb, w2_sb, dest):
                for mt in range(HT):
                    for c0, c1 in chunks:
                        n = c1 - c0
                        ps3 = psum.tile([128, QC], f32, name="g3_ps", tag="mm")
                        for kt in range(KT):
                            nc.tensor.matmul(
                                ps3[:, :n], w3_sb[:, kt, mt * 128:(mt + 1) * 128],
                                xn[:, kt, c0:c1],
                                start=(kt == 0), stop=(kt == KT - 1))
                        ps1 = psum.tile([128, QC], f32, name="g1_ps", tag="mm")
                        for kt in range(KT):
                            nc.tensor.matmul(
                                ps1[:, :n], w1_sb[:, kt, mt * 128:(mt + 1) * 128],
                                xn[:, kt, c0:c1],
                                start=(kt == 0), stop=(kt == KT - 1))
                        nc.scalar.activation(
                            out=gdest[:, mt, c0:c1], in_=ps1[:, :n],
                            func=mybir.ActivationFunctionType.Silu)
                        nc.vector.tensor_mul(
                            out=gdest[:, mt, c0:c1], in0=gdest[:, mt, c0:c1],
                            in1=ps3[:, :n])
                accum_proj(w2_sb, gdest, HT, dest, chunks)

            def mlp_thin(xn, gdest, w1_sb, w3_sb, w2_sb, dest):
                """All HT hidden tiles batched into [128, HT*NQ] psums."""
                HB = HT * NQ  # 176
                gflat = gdest.rearrange("p h q -> p (h q)")
                ps3 = psum.tile([128, HB], f32, name="g3_ps", tag="mm")
                for mt in range(HT):
                    for kt in range(KT):
                        nc.tensor.matmul(
                            ps3[:, mt * NQ:(mt + 1) * NQ],
                            w3_sb[:, kt, mt * 128:(mt + 1) * 128],
                            xn[:, kt, :],
                            start=(kt == 0), stop=(kt == KT - 1))
                g3c = scr.tile([128, HB], bf16, name="g3cq")
                nc.vector.tensor_copy(out=g3c[:], in_=ps3[:])
                ps1 = psum.tile([128, HB], f32, name="g1_ps", tag="mm")
                for mt in range(HT):
                    for kt in range(KT):
                        nc.tensor.matmul(
                            ps1[:, mt * NQ:(mt + 1) * NQ],
                            w1_sb[:, kt, mt * 128:(mt + 1) * 128],
                            xn[:, kt, :],
                            start=(kt == 0), stop=(kt == KT - 1))
                nc.scalar.activation(out=gflat[:], in_=ps1[:],
                                     func=mybir.ActivationFunctionType.Silu)
                nc.vector.tensor_mul(out=gflat[:], in0=gflat[:], in1=g3c[:])
                accum_proj(w2_sb, gdest, HT, dest, THIN_CH)

            def gather_q(dest, src):
                """dest[:, kt, 0]=src col 507; dest[:, kt, 1+3l+j]=src col 508+4l+j"""
                for kt in range(KT):
                    nc.vector.tensor_copy(out=dest[:, kt, 0:1],
                                          in_=src[:, kt, T - 1:T])
                    nc.vector.tensor_copy(
                        out=dest[:, kt, 1:NQ].rearrange("p (l s) -> p l s", s=3),
                        in_=src[:, kt, T:T + SUF].rearrange(
                            "p (l s) -> p l s", s=LBL)[:, :, 0:3])

            FULL_CH = ((0, QC), (QC, NTOK))
            THIN_CH = ((0, NQ),)

            # ================= transformer =================
            for l in range(NL):
                full = l < NL - 1
                wq_sb = wpool.tile([128, KT, D], bf16, name="wq_sb")
                wk_sb = wpool.tile([128, KT, D], bf16, name="wk_sb")
                wv_sb = wpool.tile([128, KT, D], bf16, name="wv_sb")
                wo_sb = wpool.tile([128, KT, D], bf16, name="wo_sb")
                w1_sb = wpool.tile([128, KT, HID], bf16, name="w1_sb")
                w3_sb = wpool.tile([128, KT, HID], bf16, name="w3_sb")
                w2_sb = wpool.tile([128, HT, D], bf16, name="w2_sb")
                for wsb, wd in ((wk_sb, wk), (wv_sb, wv), (wq_sb, wq),
                                (wo_sb, wo), (w1_sb, w1), (w3_sb, w3)):
                    nc.sync.dma_start(
                        out=wsb[:], in_=wd[l].rearrange("(k p) n -> p k n", p=128))
                nc.sync.dma_start(
                    out=w2_sb[:], in_=w2[l].rearrange("(k p) n -> p k n", p=128))
                if l == 0 and USE_FP8:
                    # prefetch the whole fp8 lm_head shard while the
                    # transformer runs; 32KB contiguous per-partition runs
                    # avoid the sub-512B DMA rate penalty.
                    for pp in range(2):
                        nc.sync.dma_start(out=lsb_all[:, pp],
                                          in_=lmh.rearrange("a p s v -> p a s v")[:, pp])

                xn = scr.tile([128, KT, NTOK], bf16, name="xn", bufs=1)
                rms(h, xn, NTOK, FULL_CH)
                # k/v always full (all tokens are keys); q right after k so
                # attention can begin before the v projection finishes
                proj_T(kTt, wk_sb, xn, FULL_CH, rope=True, ctb=C128, stb=S128)
                if l < NL - 1:
                    proj_T(qT, wq_sb, xn, FULL_CH, rope=True, ctb=C128, stb=S128)
                for mt in range(TT):
                    mr = _tok_rows(mt)
                    ps = psum.tile([128, D], f32, name="v_ps", tag="mm")
                    for kt in range(KT):
                        nc.tensor.matmul(
                            ps[:mr, :], xn[:, kt, mt * 128:mt * 128 + mr],
                            wv_sb[:, kt, :],
                            start=(kt == 0), stop=(kt == KT - 1))
                    nc.scalar.copy(
                        out=v_aug.rearrange("p t (h c) -> p t h c", c=HD + 1)[
                            :mr, mt, :, :HD],
                        in_=ps.rearrange("p (h c) -> p h c", c=HD)[:mr, :, :])

                if full:
                    attention_full()
                    accum_proj(wo_sb, oT, KT, h, FULL_CH)
                    xn2 = scr.tile([128, KT, NTOK], bf16, name="xn", bufs=1)
                    rms(h, xn2, NTOK, FULL_CH)
                    mlp(xn2, g1, FULL_CH, w1_sb, w3_sb, w2_sb, h)
                else:
                    gather_q(hq, h)
                    xnq = scr.tile([128, KT, NQ], bf16, name="xnq")
                    gather_q(xnq, xn)
                    proj_T(qTq, wq_sb, xnq, THIN_CH, rope=True, ctb=Cq, stb=Sq)
                    if BATCH_THIN:
                        attention_thin_batched()
                    else:
                        attention_thin_unbatched()
                    accum_proj(wo_sb, oTq, KT, hq, THIN_CH)
                    xnq2 = scr.tile([128, KT, NQ], bf16, name="xnq2")
                    rms(hq, xnq2, NQ, THIN_CH)
                    if BATCH_THIN:
                        mlp_thin(xnq2, g1q, w1_sb, w3_sb, w2_sb, hq)
                    else:
                        mlp(xnq2, g1q, THIN_CH, w1_sb, w3_sb, w2_sb, hq)

            # ============ final norm + extract + AllGather ============
            xnf = scr.tile([128, KT, NQ], bf16, name="xnf")
            rms(hq, xnf, NQ, THIN_CH)
            hsT_own = acts.tile([128, KT, NSEL], bf16, name="hsT_own")
            for kt in range(KT):
                for ll in range(NLAB):
                    nc.scalar.copy(
                        out=hsT_own[:, kt, ll * LBL:ll * LBL + 1],
                        in_=xnf[:, kt, 0:1])
                nc.scalar.copy(
                    out=hsT_own.rearrange("p k (l s) -> p k l s", s=LBL)[
                        :, kt, :, 1:LBL],
                    in_=xnf[:, kt, 1:NQ].rearrange("p (l s) -> p l s", s=3))

            cc_in = dram.tile([D, NSEL], bf16)
            cc_out = dram.tile([NCORES * D, NSEL], bf16)
            nc.sync.dma_start(
                out=cc_in.rearrange("(k p) c -> p k c", p=128), in_=hsT_own[:])
            if use_collective:
                nc.gpsimd.collective_compute(
                    "AllGather",
                    mybir.AluOpType.bypass,
                    replica_groups=[list(range(NCORES))],
                    ins=[cc_in.opt()],
                    outs=[cc_out.opt()],
                )
            else:  # timeline-sim variant: emulate with local copies
                for r in range(NCORES):
                    nc.sync.dma_start(
                        out=cc_out[r * D:(r + 1) * D, :], in_=cc_in[:])

            # hsT_all: [128, B, KT, NSEL] from ranks 0..3 of the gather
            # (B-major matches the DRAM rank order so one DMA suffices);
            # hsT_k is the kt-major copy whose 2D slices feed matmul lhsT.
            hsT_all = acts.tile([128, B, KT, NSEL], bf16, name="hsT_all")
            cc_view = cc_out.rearrange("(b k p) c -> p b k c", b=NCORES, p=128)
            nc.sync.dma_start(out=hsT_all[:], in_=cc_view[:, 0:B])
            hsT_k = acts.tile([128, KT, B * NSEL], bf16, name="hsT_k")
            for kt in range(KT):
                nc.vector.tensor_copy(
                    out=hsT_k[:, kt].rearrange("p (b c) -> p b c", b=B),
                    in_=hsT_all[:, :, kt, :])

            # ================= lm_head phase =================
            lmsel_sb = consts.tile([128, KT, NSEL], bf16)
            nc.sync.dma_start(
                out=lmsel_sb[:], in_=lmsel.rearrange("(k p) c -> p k c", p=128))
            se_sb = acts.tile([NROW, LNVCH], f32, name="se_sb")
            if USE_FP8:
                # fp8 DoubleRow copies of hs: [128, pass, 2, 80]
                hs8 = acts.tile([128, 2, 2, B * NSEL], fp8, name="hs8")
                nc.vector.tensor_scalar_mul(
                    out=hs8.rearrange("p a s c -> p (a s) c"),
                    in0=hsT_k[:],
                    scalar1=HS_SCALE)
            for j in range(LNVCH):
                if USE_FP8:
                    pl = psum.tile([NROW, LVCH], f32, name="lm_ps", tag="mm")
                    for pp in range(2):
                        nc.tensor.matmul(
                            pl[:], hs8[:, pp],
                            lsb_all[:, pp, :, j * LVCH:(j + 1) * LVCH],
                            start=(pp == 0), stop=(pp == 1),
                            perf_mode=mybir.MatmulPerfMode.DoubleRow)
                    escale = 1.0 / (LMH_SCALE * HS_SCALE)
                else:
                    pl = psum.tile([NROW, LVCH], f32, name="lm_ps", tag="mm")
                    lsb = lpool.tile([128, KT, LVCH], bf16, name="lsb")
                    nc.sync.dma_start(
                        out=lsb[:],
                        in_=lmh.rearrange("(k p) v -> p k v", p=128)[
                            :, :, j * LVCH:(j + 1) * LVCH])
                    for kt in range(KT):
                        nc.tensor.matmul(pl[:], hsT_all[:, :, kt, :],
                                         lsb[:, kt, :],
                                         start=(kt == 0), stop=(kt == KT - 1))
                    escale = 1.0
                esc = scr.tile([NROW, LVCH], f32, name="esc")
                nc.scalar.activation(
                    out=esc[:], in_=pl[:],
                    func=mybir.ActivationFunctionType.Exp,
                    scale=escale,
                    accum_out=se_sb[:, j:j + 1])
            nc.sync.dma_start(out=se_out[:], in_=se_sb[:])

            psel = psmall.tile([NROW, NSEL], f32, name="sel_ps", tag="small")
            for kt in range(KT):
                nc.tensor.matmul(psel[:], hsT_k[:, kt], lmsel_sb[:, kt, :],
                                 start=(kt == 0), stop=(kt == KT - 1))
            sel_sb = scr.tile([NROW, NSEL], f32, name="sel_sb")
            nc.scalar.copy(out=sel_sb[:], in_=psel[:])
            nc.sync.dma_start(out=sel_out[:], in_=sel_sb[:])

    nc.finalize()
    return nc


def _get_nc():
    if "nc" not in _CACHE:
        _CACHE["nc"] = build_nc()
    return _CACHE["nc"]


def _build_masks():
    """full mask [TT,128,NTOK] and thin mask [TT,128,NQ] over (k, q)."""
    k_idx = np.arange(TT * 128)
    kpos = np.where(k_idx < T, k_idx, 0)
    klab = np.where(k_idx < T, -1, (k_idx - T) // LBL)
    koff = np.where(k_idx < T, 0, (k_idx - T) % LBL)
    kvalid = k_idx < NTOK

    def allow(qpos, qlab, qoff):
        kp = kpos[:, None]; kl = klab[:, None]; ko = koff[:, None]
        prefix_k = kl == -1
        prefix_q = (qlab == -1)[None, :]
        a = np.where(
            prefix_q,
            prefix_k & (kp <= qpos[None, :]),
            prefix_k | ((kl == qlab[None, :]) & (ko <= qoff[None, :])),
        )
        return (a & kvalid[:, None]).astype(np.float32)

    q_idx = np.arange(NTOK)
    qpos = np.where(q_idx < T, q_idx, 0)
    qlab = np.where(q_idx < T, -1, (q_idx - T) // LBL)
    qoff = np.where(q_idx < T, 0, (q_idx - T) % LBL)
    maskd = allow(qpos, qlab, qoff).reshape(TT, 128, NTOK).astype(BF16)

    # thin queries: col 0 = token 507; col 1+3l+j = token 508+4l+j (j=0..2)
    tq = np.array([T - 1] + [T + 4 * l + j for l in range(NLAB) for j in range(3)])
    qpos = np.where(tq < T, tq, 0)
    qlab = np.where(tq < T, -1, (tq - T) // LBL)
    qoff = np.where(tq < T, 0, (tq - T) % LBL)
    mq1 = allow(qpos, qlab, qoff)                       # [TT*128, NQ]
    maskqa = np.tile(mq1, (1, NH)).reshape(TT, 128, NH * NQ).astype(BF16)
    return maskd, maskqa, tq


def _host_prep(inputs):
    """Build per-core in_maps from full inputs."""
    input_ids = np.asarray(inputs["input_ids"])
    label_ids = np.asarray(inputs["label_ids"])
    emb = np.asarray(inputs["emb"], dtype=np.float32)
    anw = np.asarray(inputs["attn_norm_w"], dtype=np.float32)
    fnw = np.asarray(inputs["ffn_norm_w"], dtype=np.float32)
    finw = np.asarray(inputs["final_norm_w"], dtype=np.float32)
    lm_head = np.asarray(inputs["lm_head"], dtype=np.float32)

    # fold norm weights into the consuming matmuls
    wq = np.asarray(inputs["wq"], np.float32) * anw[:, :, None]
    wk = np.asarray(inputs["wk"], np.float32) * anw[:, :, None]
    wv = np.asarray(inputs["wv"], np.float32) * anw[:, :, None]
    wo = np.asarray(inputs["wo"], np.float32)
    w1 = np.asarray(inputs["w1"], np.float32) * fnw[:, :, None]
    w3 = np.asarray(inputs["w3"], np.float32) * fnw[:, :, None]
    w2 = np.asarray(inputs["w2"], np.float32)
    lmh_f = lm_head * finw[:, None]

    suf_ids = label_ids.reshape(-1)  # (l, j) order

    # RoPE tables: packed col -> position
    pos = np.concatenate(
        [np.arange(T), np.tile(T + np.arange(LBL), NLAB)]).astype(np.float32)
    freqs = 1.0 / (10000.0 ** (np.arange(HALF, dtype=np.float32) / HALF))

    def rope_tabs(positions):
        ang = positions[None, :] * freqs[:, None]
        c = np.tile(np.cos(ang), (4, 1)).astype(BF16)
        s32 = np.sin(ang)
        s = np.concatenate([-s32, s32, -s32, s32], 0).astype(BF16)
        return c, s

    ctab, stab = rope_tabs(pos)
    maskd, maskqa, tq = _build_masks()
    cqt, sqt = rope_tabs(pos[tq])

    sigma = np.arange(128)
    sigma = (sigma // 64) * 64 + ((sigma % 64 + 32) % 64)
    permm = np.zeros((128, 128), dtype=np.float32)
    permm[sigma, np.arange(128)] = 1.0
    permm = permm.astype(BF16)

    sel_cols = suf_ids.astype(np.int64)
    lmsel = np.ascontiguousarray(lmh_f[:, sel_cols]).astype(BF16)

    common = dict(
        wq=wq.astype(BF16), wk=wk.astype(BF16), wv=wv.astype(BF16),
        wo=wo.astype(BF16), w1=w1.astype(BF16), w3=w3.astype(BF16),
        w2=w2.astype(BF16), ctab=ctab, stab=stab, cqtab=cqt, sqtab=sqt,
        maskd=maskd, maskq=maskqa, lmsel=lmsel, permd=permm,
    )
    if USE_FP8:
        # fp8 DoubleRow layout: [pass, 128, 2, V] with K row (a*2+s)*128+p
        FP8 = np.dtype(ml_dtypes.float8_e4m3)
        lmh8 = (lmh_f * LMH_SCALE).astype(FP8).reshape(2, 2, 128, V)
        lmh8 = np.ascontiguousarray(lmh8.transpose(0, 2, 1, 3))
        shards = [np.ascontiguousarray(lmh8[:, :, :, c * VSH:(c + 1) * VSH])
                  for c in range(NCORES)]
    else:
        lmh_bf = lmh_f.astype(BF16)
        shards = [np.ascontiguousarray(lmh_bf[:, c * VSH:(c + 1) * VSH])
                  for c in range(NCORES)]

    in_maps = []
    for c in range(NCORES):
        b = c % B
        tok = np.concatenate([input_ids[b], suf_ids])
        x0 = emb[tok]                      # [528, 512] fp32
        m = dict(common)
        m["x0T"] = np.ascontiguousarray(x0.T.astype(BF16) if H_BF16 else x0.T)
        m["lmh"] = shards[c]
        in_maps.append(m)
    return in_maps


def _host_combine(results):
    """Combine per-core partial sumexp + selected logits into [B, NLAB]."""
    se = np.zeros((NROW,), dtype=np.float64)
    for c in range(NCORES):
        se += np.asarray(results[c]["se_out"], np.float64).sum(axis=1)
    lse = np.log(se)  # [80], rows ordered (b, l, j)
    sel = np.asarray(results[0]["sel_out"], np.float64)  # [80, 20]
    rows = np.arange(NROW)
    bb = rows // (NLAB * LBL)
    ll = (rows % (NLAB * LBL)) // LBL
    jj = rows % LBL
    lp = sel[rows, ll * LBL + jj] - lse  # [80]
    out = np.zeros((B, NLAB), dtype=np.float64)
    np.add.at(out, (bb, ll), lp)
    return out.astype(np.float32)


def kernel(**inputs):
    nc = _get_nc()
    in_maps = _host_prep(inputs)
    res = run_bass_kernel_spmd(
        nc, in_maps, core_ids=list(range(NCORES)),
        trace=_CACHE.get("trace", False),
    )
    _CACHE["last_results"] = res
    return _host_combine(res.results)

